# revision 13
# baseline (speedup 1.0000x reference)
"""Trainium2 Bass kernel for nn_ModelNew_78847009620052 (dense_mlp).

Computes, for x [4096, 8192] and weight [8192, 8192]:
    out[b, 0] = 0.75 * sum_i x[b, i] * (sum_j weight[j, i])
(which equals 1.5 * sum(x @ W.T / 2, axis=1, keepdims=True)).

Sharding: column-shard the contraction dim IN=8192 into 8 chunks of 1024.
Core d receives x[:, d*1024:(d+1)*1024] and weight[:, d*1024:(d+1)*1024],
produces a partial [4096, 1]; host sums the 8 partials.

The kernel is DMA-bound (48 MiB of input per core; chip HBM delivers
~2.9 TB/s across the 8 cores, so ~370 GB/s/core when all queues stay
pressured). v2 layout changes vs the original:

  * Host-side repack to partition-major [128, T, 1024] for both inputs
    (row assignment is free for weight since rows are summed; for x the
    (p, i) layout matches the output store order). One DMA then moves a
    [128, G, 1024] group whose per-partition source bytes are G*4KiB
    CONTIGUOUS - large descriptors, few instructions, few semaphores.
  * Weight stream: 15x 2MiB + [2,1,1] tail groups (descending so the
    column-sum finishes ~2us after the last weight byte lands).
  * x stream: [1,1,2,4x6,2,1,1] - small leading groups so phase-2
    compute starts as soon as the weight stream ends, small trailing
    groups to keep the post-stream drain to one tile.
  * Everything issues on the sync-engine HWDGE queue in stream order;
    deep tile pools (5 x 2MiB per stream) keep the ring fed across
    buffer-recycle waits.

Per-core device algorithm:
  Phase 1: per weight group, tree-add the 2MiB group to one [128, 1024]
           tile on VectorE, then matmul with an all-ones [128, 128]
           stationary into PSUM (accumulating across groups) - this both
           reduces over the partition axis and broadcasts the column
           sums to all 128 partitions. ScalarE folds the 0.75 scale
           while moving PSUM -> SBUF.
  Phase 2: per x row-tile [128, 1024]: VectorE multiply against the
           broadcast column sums into PSUM; reduce along the free dim on
           ScalarE via activation(Copy, accum_out=...) (2 late tiles on
           VectorE to balance the two engines' drain). Results collect
           in SBUF [128, 32], transposed on TensorE so the store is one
           contiguous 16KiB DMA.

(tensor_tensor_reduce would fuse phase 2 into one VectorE op, but that
opcode crashes the device on this HW/NRT path - validated by bisection.)
"""

import numpy as np

B, IN, HID = 4096, 8192, 8192
N_CORES = 8
CHUNK = IN // N_CORES          # 1024 columns per core
SCALE = 1.5 / 2.0              # 0.75
P = 128                        # partitions
W_TILES = HID // P             # 64 weight row-tiles per core
X_TILES = B // P               # 32 x row-tiles per core

# Phase-1 reduction units: each inner list is the DMA-group sizes that are
# pre-reduced on VectorE to ONE [128, 1024] tile before the 2-matmul PSUM
# accumulation. 8-tile units keep the serial PE chain (~3us/unit even at
# the HAM-cold 1.2 GHz clock) well under the unit's ~10-11us DMA window -
# with 4-tile units the chain (~5.6us/group cold) sat right at the group's
# ~5us DMA time and any PE clock-gate jitter stalled wpool recycling and
# drained the DMA ring (observed as 35-125 GB/s bins mid-stream).
# Tail groups descend (2MiB..512KiB) and are folded into ONE unit so the
# phase-1 critical path after the last weight byte is a single cross-add
# plus one 2-matmul PSUM accumulation (~3.5us warm).
W_UNITS = [[4, 4]] * 7 + [[4, 2, 1, 1]]        # 64 tiles, 18 DMAs
# The first 4 x tiles (2 MiB) are issued into the sync FIFO right after the
# first weight unit: on a bandwidth-starved core the weight-DMA *issues* are
# paced by wpool recycling, so x DMAs queued strictly after all weight DMAs
# only start landing ~10us after the weight stream ends - phase 2 then sits
# idle waiting for data (observed 11us hole). Prefetching 4 tiles covers
# exactly that hole (4 tiles x ~1.33us compute while the first post-weight
# group lands); it costs the weight stream 2 MiB / ~BW (~5us later column
# sums) on fast cores, which their compute-bound slack absorbs.
X_EARLY = [1, 1, 2]                            # 4 tiles, 3 DMAs
X_GROUPS = [4] * 6 + [2, 1, 1]                 # 28 tiles, 9 DMAs
assert sum(sum(u) for u in W_UNITS) == W_TILES
assert sum(X_EARLY) + sum(X_GROUPS) == X_TILES

_compiled_nc = None


def _build_nc():
    import concourse.bass as bass
    import concourse.tile as tile
    from concourse import bacc, mybir

    f32 = mybir.dt.float32
    nc = bacc.Bacc(
        "TRN2",
        target_bir_lowering=False,
        debug=False,
        num_devices=N_CORES,
    )

    x_d = nc.dram_tensor("x", [P, X_TILES, CHUNK], f32, kind="ExternalInput")
    w_d = nc.dram_tensor("w", [P, W_TILES, CHUNK], f32, kind="ExternalInput")
    out_d = nc.dram_tensor("out", [B, 1], f32, kind="ExternalOutput")

    with tile.TileContext(nc) as tc:
        with (
            tc.tile_pool(name="wpool", bufs=6) as wpool,
            tc.tile_pool(name="xpool", bufs=5) as xpool,
            tc.tile_pool(name="const", bufs=1) as const,
            tc.tile_pool(name="psum", bufs=1, space="PSUM") as psum_pool,
        ):
            from concourse.masks import make_identity

            ones = const.tile([P, P], f32)
            nc.vector.memset(ones[:], 1.0)
            identity = const.tile([P, P], f32)
            make_identity(nc, identity)

            # Phase 1: column sums of the weight chunk over all 8192 rows.
            psum_bc = psum_pool.tile([P, CHUNK], f32, tag="psum_bc")  # 2 banks
            xearly = []
            pos = 0
            for ui, unit in enumerate(W_UNITS):
                if ui == 1:
                    # early x prefetch into the same sync FIFO (see X_EARLY)
                    xpos = 0
                    for g in X_EARLY:
                        xt = xpool.tile([P, 4, CHUNK], f32, tag="xtile")
                        nc.sync.dma_start(xt[:, :g, :], x_d[:, xpos : xpos + g, :])
                        xearly.append((xt, xpos, g))
                        xpos += g
                wts = []
                for g in unit:
                    wt = wpool.tile([P, 4, CHUNK], f32, tag="wtile")
                    nc.sync.dma_start(wt[:, :g, :], w_d[:, pos : pos + g, :])
                    # in-group tree reduce on VectorE -> wt[:, 0, :]
                    if g == 4:
                        nc.vector.tensor_add(wt[:, 0, :], wt[:, 0, :], wt[:, 1, :])
                        nc.vector.tensor_add(wt[:, 2, :], wt[:, 2, :], wt[:, 3, :])
                        nc.vector.tensor_add(wt[:, 0, :], wt[:, 0, :], wt[:, 2, :])
                    elif g == 2:
                        nc.vector.tensor_add(wt[:, 0, :], wt[:, 0, :], wt[:, 1, :])
                    wts.append(wt)
                    pos += g
                if ui == len(W_UNITS) - 1:
                    # PE clock-gate pre-warm: ~4us of dummy matmuls into a
                    # scratch bank, gated on the tail unit's first (2MiB)
                    # group landing ~5us before the stream ends. They read
                    # wt[:,1,:], which the in-group adds only read - no
                    # hazard, no delay to the reduction chain. The HAM
                    # up-clocks the PE 1.2->2.4 GHz after ~3.4us of
                    # sustained activity, so the real tail accumulation
                    # chain (phase-1 critical path after the last weight
                    # byte) runs warm.
                    warm = psum_pool.tile([P, 512], f32, tag="psum_warm")
                    for _ in range(3):
                        nc.tensor.matmul(
                            warm[:], ones[:], wts[0][:, 1, 0:512],
                            start=True, stop=True,
                        )
                # Cross-group reduce into a dedicated unit-sum tile. The
                # final add targets usum (not wt in-place) so the weight
                # buffers are released at VectorE speed - the PE matmul
                # chain (~5.8us/unit at the cold clock) then never gates
                # wpool recycling / DMA issue; its jitter is absorbed by
                # the 3-deep usum pool. (With in-place folding, a starved
                # core's issue cadence stretched until the ring emptied -
                # observed 21 GB/s bins and a positive-feedback collapse.)
                usum = wpool.tile([P, CHUNK], f32, tag="usum", bufs=3)
                nc.vector.tensor_add(
                    usum[:], wts[0][:, 0, :], wts[1][:, 0, :]
                )
                for other in wts[2:]:
                    nc.vector.tensor_add(usum[:], usum[:], other[:, 0, :])
                for h in range(2):
                    nc.tensor.matmul(
                        psum_bc[:, h * 512 : (h + 1) * 512],
                        ones[:],
                        usum[:, h * 512 : (h + 1) * 512],
                        start=(ui == 0),
                        stop=(ui == len(W_UNITS) - 1),
                    )

            # Broadcast column sums now live in every PSUM partition; move to
            # SBUF on ScalarE (folding in the 0.75 scale) so VectorE stays
            # free for phase 2.
            w_bcast = const.tile([P, CHUNK], f32)
            nc.scalar.mul(w_bcast[:], psum_bc[:], SCALE)

            # Phase 2: multiply + reduce of x tiles against w_bcast.
            # ScalarE activation(Copy, accum_out) does most row-reductions
            # (reads PSUM at its lower base cost); VectorE takes two late
            # tiles to balance the engines' drain after the stream ends.
            # (A GpSimd-assisted variant was tried and measured SLOWER:
            # POOL tensor_mul is ~2.4-3.3us/tile and its concurrent reads
            # of the shared w_bcast operand degraded VectorE muls ~2.4x.)
            DVE_REDUCE = {29, 31}
            s_sbuf = const.tile([P, X_TILES], f32)
            scratch = const.tile([P, CHUNK], f32)

            def do_tile(i, src):
                prod = psum_pool.tile([P, CHUNK], f32, tag="prodps", bufs=2)
                nc.vector.tensor_mul(prod[:], src, w_bcast[:])
                if i in DVE_REDUCE:
                    nc.vector.reduce_sum(
                        s_sbuf[:, i : i + 1], prod[:], axis=mybir.AxisListType.X
                    )
                else:
                    nc.scalar.activation(
                        scratch[:],
                        prod[:],
                        mybir.ActivationFunctionType.Copy,
                        bias=0.0,
                        scale=1.0,
                        accum_out=s_sbuf[:, i : i + 1],
                    )

            for xt, xpos, g in xearly:
                for j in range(g):
                    do_tile(xpos + j, xt[:, j, :])
            pos = sum(X_EARLY)
            for g in X_GROUPS:
                xt = xpool.tile([P, 4, CHUNK], f32, tag="xtile")
                nc.sync.dma_start(xt[:, :g, :], x_d[:, pos : pos + g, :])
                for j in range(g):
                    do_tile(pos + j, xt[:, j, :])
                pos += g

            # Transpose s_sbuf [128, 32] -> [32, 128] on TensorE so the store
            # is contiguous 512B runs in DRAM (a [128, 32]-layout store would
            # shatter into 4096 4-byte DMA packets - measured 16us).
            psum_t = psum_pool.tile([X_TILES, P], f32, tag="psum_t")
            nc.tensor.transpose(psum_t[:], s_sbuf[:], identity[:])
            sT = const.tile([X_TILES, P], f32)
            nc.scalar.copy(sT[:], psum_t[:])
            # out[n*128 + p, 0] = sT[n, p]
            out_ap = out_d[:].rearrange("(n p) o -> n (p o)", p=P)
            nc.sync.dma_start(out_ap, sT[:])

    nc.compile()
    return nc


def _get_nc():
    global _compiled_nc
    if _compiled_nc is None:
        _compiled_nc = _build_nc()
    return _compiled_nc


def _shard_inputs(x: np.ndarray, weight: np.ndarray):
    """Column-shard both tensors and repack each shard partition-major
    ([128, tiles, 1024]) so every DMA descriptor covers contiguous DRAM."""
    in_maps = []
    for d in range(N_CORES):
        xc = x[:, d * CHUNK : (d + 1) * CHUNK]
        wc = weight[:, d * CHUNK : (d + 1) * CHUNK]
        xr = np.ascontiguousarray(
            xc.reshape(X_TILES, P, CHUNK).transpose(1, 0, 2)
        )
        wr = np.ascontiguousarray(
            wc.reshape(W_TILES, P, CHUNK).transpose(1, 0, 2)
        )
        in_maps.append({"x": xr, "w": wr})
    return in_maps


def kernel(x: np.ndarray, weight: np.ndarray) -> np.ndarray:
    from concourse.bass_utils import run_bass_kernel_spmd

    x = np.asarray(x, dtype=np.float32)
    weight = np.asarray(weight, dtype=np.float32)
    assert x.shape == (B, IN) and weight.shape == (HID, IN)

    nc = _get_nc()
    in_maps = _shard_inputs(x, weight)
    res = run_bass_kernel_spmd(nc, in_maps, core_ids=list(range(N_CORES)))
    acc = np.zeros((B, 1), dtype=np.float64)
    for d in range(N_CORES):
        acc += res.results[d]["out"].astype(np.float64)
    return acc.astype(np.float32)


# revision 17
# speedup vs baseline: 1.0033x; 1.0033x over previous
"""Trainium2 Bass kernel for nn_ModelNew_78847009620052 (dense_mlp).

Computes, for x [4096, 8192] and weight [8192, 8192]:
    out[b, 0] = 0.75 * sum_i x[b, i] * (sum_j weight[j, i])
(which equals 1.5 * sum(x @ W.T / 2, axis=1, keepdims=True)).

Sharding: column-shard the contraction dim IN=8192 into 8 chunks of 1024.
Core d receives x[:, d*1024:(d+1)*1024] and weight[:, d*1024:(d+1)*1024],
produces a partial [4096, 1]; host sums the 8 partials.

The kernel is DMA-bound (48 MiB of input per core; chip HBM delivers
~2.9 TB/s across the 8 cores, so ~370 GB/s/core when all queues stay
pressured). v2 layout changes vs the original:

  * Host-side repack to partition-major [128, T, 1024] for both inputs
    (row assignment is free for weight since rows are summed; for x the
    (p, i) layout matches the output store order). One DMA then moves a
    [128, G, 1024] group whose per-partition source bytes are G*4KiB
    CONTIGUOUS - large descriptors, few instructions, few semaphores.
  * Weight stream: 15x 2MiB + [2,1,1] tail groups (descending so the
    column-sum finishes ~2us after the last weight byte lands).
  * x stream: [1,1,2,4x6,2,1,1] - small leading groups so phase-2
    compute starts as soon as the weight stream ends, small trailing
    groups to keep the post-stream drain to one tile.
  * Everything issues on the sync-engine HWDGE queue in stream order;
    deep tile pools (5 x 2MiB per stream) keep the ring fed across
    buffer-recycle waits.

Per-core device algorithm:
  Phase 1: per weight group, tree-add the 2MiB group to one [128, 1024]
           tile on VectorE, then matmul with an all-ones [128, 128]
           stationary into PSUM (accumulating across groups) - this both
           reduces over the partition axis and broadcasts the column
           sums to all 128 partitions. ScalarE folds the 0.75 scale
           while moving PSUM -> SBUF.
  Phase 2: per x row-tile [128, 1024]: VectorE multiply against the
           broadcast column sums into PSUM; reduce along the free dim on
           ScalarE via activation(Copy, accum_out=...) (2 late tiles on
           VectorE to balance the two engines' drain). Results collect
           in SBUF [128, 32], transposed on TensorE so the store is one
           contiguous 16KiB DMA.

(tensor_tensor_reduce would fuse phase 2 into one VectorE op, but that
opcode crashes the device on this HW/NRT path - validated by bisection.)
"""

import numpy as np

B, IN, HID = 4096, 8192, 8192
N_CORES = 8
CHUNK = IN // N_CORES          # 1024 columns per core
SCALE = 1.5 / 2.0              # 0.75
P = 128                        # partitions
W_TILES = HID // P             # 64 weight row-tiles per core
X_TILES = B // P               # 32 x row-tiles per core

# Phase-1 reduction units: each inner list is the DMA-group sizes that are
# pre-reduced on VectorE to ONE [128, 1024] tile before the 2-matmul PSUM
# accumulation. 8-tile units keep the serial PE chain (~3us/unit even at
# the HAM-cold 1.2 GHz clock) well under the unit's ~10-11us DMA window -
# with 4-tile units the chain (~5.6us/group cold) sat right at the group's
# ~5us DMA time and any PE clock-gate jitter stalled wpool recycling and
# drained the DMA ring (observed as 35-125 GB/s bins mid-stream).
# Tail groups descend (2MiB..512KiB) and are folded into ONE unit so the
# phase-1 critical path after the last weight byte is a single cross-add
# plus one 2-matmul PSUM accumulation (~3.5us warm).
W_UNITS = [[4, 4]] * 7 + [[4, 2, 1, 1]]        # 64 tiles, 18 DMAs
# The first 4 x tiles (2 MiB) are issued into the sync FIFO right after the
# first weight unit: on a bandwidth-starved core the weight-DMA *issues* are
# paced by wpool recycling, so x DMAs queued strictly after all weight DMAs
# only start landing ~10us after the weight stream ends - phase 2 then sits
# idle waiting for data (observed 11us hole). Prefetching 4 tiles covers
# exactly that hole (4 tiles x ~1.33us compute while the first post-weight
# group lands); it costs the weight stream 2 MiB / ~BW (~5us later column
# sums) on fast cores, which their compute-bound slack absorbs.
X_EARLY = [1, 1, 2]                            # 4 tiles, 3 DMAs
X_GROUPS = [4] * 6 + [2, 1, 1]                 # 28 tiles, 9 DMAs
assert sum(sum(u) for u in W_UNITS) == W_TILES
assert sum(X_EARLY) + sum(X_GROUPS) == X_TILES

_compiled_nc = None


def _build_nc():
    import concourse.bass as bass
    import concourse.tile as tile
    from concourse import bacc, mybir

    f32 = mybir.dt.float32
    nc = bacc.Bacc(
        "TRN2",
        target_bir_lowering=False,
        debug=False,
        num_devices=N_CORES,
    )

    x_d = nc.dram_tensor("x", [P, X_TILES, CHUNK], f32, kind="ExternalInput")
    w_d = nc.dram_tensor("w", [P, W_TILES, CHUNK], f32, kind="ExternalInput")
    out_d = nc.dram_tensor("out", [B, 1], f32, kind="ExternalOutput")

    with tile.TileContext(nc) as tc:
        with (
            tc.tile_pool(name="wpool", bufs=6) as wpool,
            tc.tile_pool(name="xpool", bufs=5) as xpool,
            tc.tile_pool(name="const", bufs=1) as const,
            tc.tile_pool(name="psum", bufs=1, space="PSUM") as psum_pool,
        ):
            from concourse.masks import make_identity

            # 0.75 output scale folded into the reduction stationary: the
            # broadcast column sums land in PSUM already scaled, and phase-2
            # muls read them straight from PSUM - no PSUM->SBUF copy on the
            # phase-boundary critical path.
            ones = const.tile([P, P], f32)
            nc.vector.memset(ones[:], SCALE)
            identity = const.tile([P, P], f32)
            make_identity(nc, identity)

            # Phase 1: column sums of the weight chunk over all 8192 rows.
            psum_bc = psum_pool.tile([P, CHUNK], f32, tag="psum_bc")  # 2 banks
            xearly = []
            pos = 0
            for ui, unit in enumerate(W_UNITS):
                if ui == 1:
                    # early x prefetch into the same sync FIFO (see X_EARLY)
                    xpos = 0
                    for g in X_EARLY:
                        xt = xpool.tile([P, 4, CHUNK], f32, tag="xtile")
                        nc.sync.dma_start(xt[:, :g, :], x_d[:, xpos : xpos + g, :])
                        xearly.append((xt, xpos, g))
                        xpos += g
                wts = []
                for g in unit:
                    wt = wpool.tile([P, 4, CHUNK], f32, tag="wtile")
                    nc.sync.dma_start(wt[:, :g, :], w_d[:, pos : pos + g, :])
                    # in-group tree reduce on VectorE -> wt[:, 0, :]
                    if g == 4:
                        nc.vector.tensor_add(wt[:, 0, :], wt[:, 0, :], wt[:, 1, :])
                        nc.vector.tensor_add(wt[:, 2, :], wt[:, 2, :], wt[:, 3, :])
                        nc.vector.tensor_add(wt[:, 0, :], wt[:, 0, :], wt[:, 2, :])
                    elif g == 2:
                        nc.vector.tensor_add(wt[:, 0, :], wt[:, 0, :], wt[:, 1, :])
                    wts.append(wt)
                    pos += g
                if ui == len(W_UNITS) - 1:
                    # PE clock-gate pre-warm: ~4us of dummy matmuls into a
                    # scratch bank, gated on the tail unit's first (2MiB)
                    # group landing ~5us before the stream ends. They read
                    # wt[:,1,:], which the in-group adds only read - no
                    # hazard, no delay to the reduction chain. The HAM
                    # up-clocks the PE 1.2->2.4 GHz after ~3.4us of
                    # sustained activity, so the real tail accumulation
                    # chain (phase-1 critical path after the last weight
                    # byte) runs warm.
                    warm = psum_pool.tile([P, 512], f32, tag="psum_warm")
                    for _ in range(3):
                        nc.tensor.matmul(
                            warm[:], ones[:], wts[0][:, 1, 0:512],
                            start=True, stop=True,
                        )
                # Cross-group reduce into a dedicated unit-sum tile. The
                # final add targets usum (not wt in-place) so the weight
                # buffers are released at VectorE speed - the PE matmul
                # chain (~5.8us/unit at the cold clock) then never gates
                # wpool recycling / DMA issue; its jitter is absorbed by
                # the 3-deep usum pool. (With in-place folding, a starved
                # core's issue cadence stretched until the ring emptied -
                # observed 21 GB/s bins and a positive-feedback collapse.)
                usum = wpool.tile([P, CHUNK], f32, tag="usum", bufs=3)
                nc.vector.tensor_add(
                    usum[:], wts[0][:, 0, :], wts[1][:, 0, :]
                )
                if len(wts) == 4:
                    # tree the tail: pair the last two (smallest, latest)
                    # groups first so only ONE add chains after the final
                    # weight byte lands.
                    nc.vector.tensor_add(
                        wts[2][:, 0, :], wts[2][:, 0, :], wts[3][:, 0, :]
                    )
                    nc.vector.tensor_add(usum[:], usum[:], wts[2][:, 0, :])
                else:
                    for other in wts[2:]:
                        nc.vector.tensor_add(usum[:], usum[:], other[:, 0, :])
                for h in range(2):
                    nc.tensor.matmul(
                        psum_bc[:, h * 512 : (h + 1) * 512],
                        ones[:],
                        usum[:, h * 512 : (h + 1) * 512],
                        start=(ui == 0),
                        stop=(ui == len(W_UNITS) - 1),
                    )

            # Broadcast (0.75-scaled) column sums now live in every PSUM
            # partition; phase-2 muls read them directly from PSUM.

            # Phase 2: multiply + reduce of x tiles against w_bcast.
            # ScalarE activation(Copy, accum_out) does most row-reductions
            # (reads PSUM at its lower base cost); VectorE takes two late
            # tiles to balance the engines' drain after the stream ends.
            # (A GpSimd-assisted variant was tried and measured SLOWER:
            # POOL tensor_mul is ~2.4-3.3us/tile and its concurrent reads
            # of the shared w_bcast operand degraded VectorE muls ~2.4x.)
            DVE_REDUCE = {29, 31}
            s_sbuf = const.tile([P, X_TILES], f32)
            scratch = const.tile([P, CHUNK], f32)

            def do_tile(i, src):
                prod = psum_pool.tile([P, CHUNK], f32, tag="prodps", bufs=2)
                nc.vector.tensor_mul(prod[:], src, psum_bc[:])
                if i in DVE_REDUCE:
                    nc.vector.reduce_sum(
                        s_sbuf[:, i : i + 1], prod[:], axis=mybir.AxisListType.X
                    )
                else:
                    nc.scalar.activation(
                        scratch[:],
                        prod[:],
                        mybir.ActivationFunctionType.Copy,
                        bias=0.0,
                        scale=1.0,
                        accum_out=s_sbuf[:, i : i + 1],
                    )

            for xt, xpos, g in xearly:
                for j in range(g):
                    do_tile(xpos + j, xt[:, j, :])
            pos = sum(X_EARLY)
            for g in X_GROUPS:
                xt = xpool.tile([P, 4, CHUNK], f32, tag="xtile")
                nc.sync.dma_start(xt[:, :g, :], x_d[:, pos : pos + g, :])
                for j in range(g):
                    do_tile(pos + j, xt[:, j, :])
                pos += g

            # Transpose s_sbuf [128, 32] -> [32, 128] on TensorE so the store
            # is contiguous 512B runs in DRAM (a [128, 32]-layout store would
            # shatter into 4096 4-byte DMA packets - measured 16us).
            psum_t = psum_pool.tile([X_TILES, P], f32, tag="psum_t")
            nc.tensor.transpose(psum_t[:], s_sbuf[:], identity[:])
            sT = const.tile([X_TILES, P], f32)
            nc.scalar.copy(sT[:], psum_t[:])
            # out[n*128 + p, 0] = sT[n, p]
            out_ap = out_d[:].rearrange("(n p) o -> n (p o)", p=P)
            nc.sync.dma_start(out_ap, sT[:])

    nc.compile()
    return nc


def _get_nc():
    global _compiled_nc
    if _compiled_nc is None:
        _compiled_nc = _build_nc()
    return _compiled_nc


def _shard_inputs(x: np.ndarray, weight: np.ndarray):
    """Column-shard both tensors and repack each shard partition-major
    ([128, tiles, 1024]) so every DMA descriptor covers contiguous DRAM."""
    in_maps = []
    for d in range(N_CORES):
        xc = x[:, d * CHUNK : (d + 1) * CHUNK]
        wc = weight[:, d * CHUNK : (d + 1) * CHUNK]
        xr = np.ascontiguousarray(
            xc.reshape(X_TILES, P, CHUNK).transpose(1, 0, 2)
        )
        wr = np.ascontiguousarray(
            wc.reshape(W_TILES, P, CHUNK).transpose(1, 0, 2)
        )
        in_maps.append({"x": xr, "w": wr})
    return in_maps


def kernel(x: np.ndarray, weight: np.ndarray) -> np.ndarray:
    from concourse.bass_utils import run_bass_kernel_spmd

    x = np.asarray(x, dtype=np.float32)
    weight = np.asarray(weight, dtype=np.float32)
    assert x.shape == (B, IN) and weight.shape == (HID, IN)

    nc = _get_nc()
    in_maps = _shard_inputs(x, weight)
    res = run_bass_kernel_spmd(nc, in_maps, core_ids=list(range(N_CORES)))
    acc = np.zeros((B, 1), dtype=np.float64)
    for d in range(N_CORES):
        acc += res.results[d]["out"].astype(np.float64)
    return acc.astype(np.float32)
